# revision 62
# baseline (speedup 1.0000x reference)
"""Trainium2 Bass kernel for GQA attention prefill (B=2, T=2048, D=4096, N=32, K=8, H=128).

Sharding: 8 cores = 2 (batch) x 4 (head-groups). Each core handles one batch
element, 8 q-heads and its 2 kv-heads, producing a partial output projection
(summed over its heads, bf16). Host sums the 4 partials per batch element.

Per-core pipeline, software-pipelined per head so PE never idles:
  passA(tb):  k,v projections from x_sb (x DMA'd once per t-block, kept
              resident in SBUF); rope(k) via a +-1 permutation matmul plus
              DVE elementwise with host cos/sin tables.
  per head h: q-projection matmuls for head h+1 are emitted interleaved with
              head h's attention s-block loop, so PE has dense independent
              work while ACT computes exp. Attention: logitsT [s128,t<=512]
              = kT-block @ qt (bf16, no transposes), exp on ACT, 0/1
              triangle mask multiply on DVE for diagonal tiles only, AV
              accumulates in PSUM. Softmax denominators: non-diagonal exp
              tiles accumulate on the Pool engine (off the critical path);
              one ones-vector matmul over that accumulator plus tiny per-tile
              ones-matmuls for the 4 diagonal blocks finish l on PE.
              During the last head, the next t-block's x / cos / sin DMAs
              are emitted so passA never waits on the queue.
  ph3(tb):    output projection from per-head enc tiles (no whole-tile dep),
              PSUM -> bf16 SBUF copies round-robined over DVE/Pool/ACT,
              DMA out per 512-wide d-chunk.
"""

import os
import sys

import numpy as np

for _p in ("/opt/trn_rl_repo", "/root/.axon_site/_ro/trn_rl_repo"):
    if _p not in sys.path and os.path.isdir(_p):
        sys.path.append(_p)

import ml_dtypes

BF16 = ml_dtypes.bfloat16

P = 128
T = 2048
D = 4096
H = 128
NQ = 8   # q heads per core
NKV = 2  # kv heads per core
TB = 512
NTB = T // TB        # 4
DT = D // P          # 32 d-tiles
NSB = T // P         # 16 s-blocks
TC = TB // P         # 4 t-chunks per t-block
NDC = D // TB        # 8 d-chunks for the output projection
SCALE = float(H) ** -0.5

_STATE = {}


def _build_nc():
    import concourse.mybir as mybir
    import concourse.tile as tile
    from concourse import bacc
    from concourse import bass_isa

    f32 = mybir.dt.float32
    f32r = mybir.dt.float32r
    bf16 = mybir.dt.bfloat16
    Alu = mybir.AluOpType
    Act = mybir.ActivationFunctionType

    nc = bacc.Bacc(None, target_bir_lowering=False, debug=False)

    xT = nc.dram_tensor("xT", [D, T], bf16, kind="ExternalInput")
    wq = nc.dram_tensor("wq", [D, NQ, H], bf16, kind="ExternalInput")
    wk = nc.dram_tensor("wk", [D, NKV, H], bf16, kind="ExternalInput")
    wv = nc.dram_tensor("wv", [D, NKV, H], bf16, kind="ExternalInput")
    wo = nc.dram_tensor("wo", [NQ, H, D], bf16, kind="ExternalInput")
    cos = nc.dram_tensor("cos", [P, T], bf16, kind="ExternalInput")
    sin = nc.dram_tensor("sin", [P, T], bf16, kind="ExternalInput")
    mrot = nc.dram_tensor("mrot", [P, P], bf16, kind="ExternalInput")
    tri = nc.dram_tensor("tri", [P, P], bf16, kind="ExternalInput")
    ones = nc.dram_tensor("ones", [P, 1], f32r, kind="ExternalInput")
    onesb = nc.dram_tensor("onesb", [P, 1], bf16, kind="ExternalInput")
    y = nc.dram_tensor("y", [T, D], bf16, kind="ExternalOutput")

    with tile.TileContext(nc) as tc:
        with (
            tc.tile_pool(name="const", bufs=1) as const,
            tc.tile_pool(name="xp", bufs=2) as xp,
            tc.tile_pool(name="wqp", bufs=2) as wqp,
            tc.tile_pool(name="qtp", bufs=3) as qtp,
            tc.tile_pool(name="rp", bufs=3) as rp,
            tc.tile_pool(name="ep", bufs=6) as ep,
            tc.tile_pool(name="eap", bufs=2) as eap,
            tc.tile_pool(name="encp", bufs=9) as encp,
            tc.tile_pool(name="lp", bufs=1) as lp,
            tc.tile_pool(name="wop", bufs=2) as wop,
            tc.tile_pool(name="yp", bufs=2) as yp,
            tc.tile_pool(name="ps", bufs=1, space="PSUM") as ps,
        ):
            wk_sb = const.tile([P, DT, NKV, H], bf16, tag="wk")
            wv_sb = const.tile([P, DT, NKV * H], bf16, tag="wv")
            wk_r = wk.rearrange("(dt p) h e -> p dt h e", p=P)
            wv_r = wv.rearrange("(dt p) h e -> p dt (h e)", p=P)
            tri_sb = const.tile([P, P], bf16, tag="tri")
            cos_sb = const.tile([P, T], bf16, tag="cos")
            sin_sb = const.tile([P, T], bf16, tag="sin")
            kT_all = const.tile([P, NKV, T], bf16, tag="kT")
            v_all = const.tile([P, NKV, NSB, H], bf16, tag="v")

            def x_dma_closures(tb, x_sb, tables=True):
                """8 closures, each DMA-ing a 4-d-tile chunk of x for tb
                (plus this t-block's cos/sin on the first chunk)."""
                tsl = slice(tb * TB, (tb + 1) * TB)

                def mk(c8):
                    def emit():
                        nc.sync.dma_start(
                            x_sb[:, c8 * 4:(c8 + 1) * 4, :],
                            xT[c8 * 4 * P:(c8 + 1) * 4 * P, tsl]
                            .rearrange("(g p) t -> p g t", p=P))
                        if c8 == 0 and tables:
                            nc.sync.dma_start(cos_sb[:, tsl], cos[:, tsl])
                            nc.sync.dma_start(sin_sb[:, tsl], sin[:, tsl])
                    return emit

                return [mk(c8) for c8 in range(8)]

            def rope(dst, src_ps, tb):
                """dst[:] = rope(src_ps) for one head's [H, TB] block.
                Half-swap via SBUF->SBUF DMA partition reorder (no PE);
                the sign of the swapped half is folded into the sin table
                ([-sin; +sin])."""
                cs = cos_sb[:, tb * TB:(tb + 1) * TB]
                sn = sin_sb[:, tb * TB:(tb + 1) * TB]
                raw = rp.tile([P, TB], bf16, tag="raw", name="raw")
                nc.scalar.copy(raw[:], src_ps[:])
                shuf = rp.tile([P, TB], bf16, tag="shuf", name="shuf")
                nc.sync.dma_start(shuf[0:P // 2, :], raw[P // 2:P, :])
                nc.sync.dma_start(shuf[P // 2:P, :], raw[0:P // 2, :])
                tmp = rp.tile([P, TB], bf16, tag="tmp", name="tmp")
                nc.vector.tensor_tensor(dst, raw[:], cs, Alu.mult)
                nc.vector.tensor_tensor(tmp[:], shuf[:], sn, Alu.mult)
                nc.vector.tensor_tensor(dst, dst, tmp[:], Alu.add)

            def kv_k_chunks(tbx):
                """k-projection matmul chunk closures (+ psk tiles) for tbx."""
                x_sb = x_tiles[tbx]
                psk = [ps.tile([P, TB], f32, tag="big", bufs=6,
                               name=f"psk{_k}") for _k in range(NKV)]

                def mk(d0, d1):
                    def emit():
                        for dt in range(d0, d1):
                            for kk in range(NKV):
                                nc.tensor.matmul(
                                    psk[kk][:], wk_sb[:, dt, kk, :],
                                    x_sb[:, dt, :],
                                    start=dt == 0, stop=dt == DT - 1)
                    return emit

                return [mk(d, d + 4) for d in range(0, DT, 4)], psk

            def kv_v_chunks(tbx):
                """v-projection closures for tbx. Each of the 4 t-chunks gets
                its OWN PSUM tile (one accumulation group per bank — two
                groups in one bank is illegal), sequenced so a chunk is
                copied out to v_all before its bank slot is reused."""
                x_sb = x_tiles[tbx]
                state = {}

                def mk_mm(c, d0, d1):
                    def emit():
                        if d0 == 0:
                            state[c] = ps.tile([P, NKV * H], f32, tag="big",
                                               bufs=6, name=f"psv{c}")
                        for dt in range(d0, d1):
                            nc.tensor.matmul(
                                state[c][:],
                                x_sb[:, dt, c * P:(c + 1) * P],
                                wv_sb[:, dt, :],
                                start=dt == 0, stop=dt == DT - 1)
                    return emit

                def mk_copy(c):
                    def emit():
                        nc.scalar.copy(
                            v_all[:, :, tbx * TC + c, :],
                            state[c][:].rearrange("p (h e) -> p h e", h=NKV))
                    return emit

                chunks = []
                for c in (0, 2):
                    chunks += [mk_mm(c, d, d + 8) for d in range(0, DT, 8)]
                chunks.append(mk_copy(0))
                chunks += [mk_mm(1, d, d + 8) for d in range(0, DT, 8)]
                chunks.append(mk_copy(2))
                chunks += [mk_mm(3, d, d + 8) for d in range(0, DT, 8)]
                chunks.append(mk_copy(1))
                chunks.append(mk_copy(3))
                return chunks, None

            def kv_finish_k(tbx, psk):
                tsl = slice(tbx * TB, (tbx + 1) * TB)
                for kk in range(NKV):
                    rope(kT_all[:, kk, tsl], psk[kk], tbx)

            wqs = {}
            psqs = {}

            def load_wq(h, tb):
                wq_t = wqp.tile([P, DT, H], bf16, tag="wq", name=f"wq{h}")
                for c in range(4):
                    nc.sync.dma_start(
                        wq_t[:, c * 8:(c + 1) * 8, :],
                        wq[c * 8 * P:(c + 1) * 8 * P, h, :]
                        .rearrange("(dt p) e -> p dt e", p=P))
                wqs[h] = wq_t

            def qchunks(h, tb):
                """Closures each emitting a few of head h's 32 accumulating
                q-projection matmuls (wq must already be loading)."""
                x_sb = x_tiles[tb]
                wq_t = wqs[h]
                psq = ps.tile([P, TB], f32, tag="big", bufs=6, name=f"psq{h}")
                psqs[h] = psq

                def mk(d0, d1):
                    def emit():
                        for dt in range(d0, d1):
                            nc.tensor.matmul(psq[:], wq_t[:, dt, :],
                                             x_sb[:, dt, :],
                                             start=dt == 0, stop=dt == DT - 1)
                    return emit

                return [mk(d, min(d + 4, DT)) for d in range(0, DT, 4)]

            def attn(h, tb, qt, enc_h, fills, tail_fn, pe_fills=True,
                     head_fn=None, fin_prev=None):
                """Attention for head h. `fills` are closures interleaved into
                the s-block loop (a later head's q matmuls, or DMA prefetch);
                `tail_fn` (that head's rope) is emitted once fills are done."""
                nsb = TC * (tb + 1)
                ndg = nsb - TC  # non-diagonal s-block count
                kk = h // 4
                enc_ps = ps.tile([P, TB], f32, tag="acc", bufs=2,
                                 name=f"encps{h}")
                exacc = eap.tile([P, TB], bf16, tag="eacc", name=f"eacc{h}")
                look = 3 if pe_fills else 4
                ex = [None] * nsb
                tail_state = {"done": tail_fn is None}

                def pre(sb):
                    r = sb - ndg
                    off = P * r if r >= 0 else 0
                    csl = slice(off, TB)
                    lg = ps.tile([P, TB], f32, tag="big", bufs=6,
                                 name=f"lg{sb}")
                    nc.tensor.matmul(lg[:, csl],
                                     kT_all[:, kk, sb * P:(sb + 1) * P],
                                     qt[:, csl])
                    ex_t = ep.tile([P, TB], bf16, tag="ex", name=f"ex{sb}")
                    nc.scalar.activation(ex_t[:, csl], lg[:, csl], Act.Exp,
                                         scale=SCALE)
                    if r >= 0:
                        nc.vector.tensor_tensor(ex_t[:, off:off + P],
                                                ex_t[:, off:off + P],
                                                tri_sb[:], Alu.mult)
                    ex[sb] = (ex_t, csl)

                def post(sb):
                    ex_t, csl = ex[sb]
                    nc.tensor.matmul(enc_ps[:, csl], v_all[:, kk, sb, :],
                                     ex_t[:, csl],
                                     start=sb == 0, stop=sb == nsb - 1)
                    # exp-tile accumulation for the softmax denominator (DVE)
                    if sb == 0:
                        nc.vector.tensor_copy(exacc[:], ex_t[:])
                    else:
                        nc.vector.tensor_tensor(exacc[:, csl], exacc[:, csl],
                                                ex_t[:, csl], Alu.add)

                ci = 0
                while ci < min(2, len(fills)):
                    fills[ci]()
                    ci += 1
                if head_fn is not None:
                    head_fn()
                for sb in range(min(look, nsb)):
                    pre(sb)
                if fin_prev is not None:
                    fin_prev()
                for sb in range(nsb):
                    want = min(len(fills),
                               max((sb + 3) * len(fills) // nsb, 3))
                    while ci < want:
                        fills[ci]()
                        ci += 1
                    if ci == len(fills) and not tail_state["done"]:
                        tail_fn()
                        tail_state["done"] = True
                    post(sb)
                    if sb + look < nsb:
                        pre(sb + look)
                while ci < len(fills):
                    fills[ci]()
                    ci += 1
                if not tail_state["done"]:
                    tail_fn()

                def finalize():
                    lsum = lp.tile([P, TB], f32, tag="lsum", name="lsum")
                    nc.gpsimd.partition_all_reduce(lsum[:], exacc[:], P,
                                                   bass_isa.ReduceOp.add)
                    rinv = lp.tile([P, TB], f32, tag="rinv", name="rinv")
                    nc.vector.reciprocal(rinv[:], lsum[:])
                    nc.vector.tensor_tensor(enc_h[:], enc_ps[:], rinv[:],
                                            Alu.mult)
                return finalize

            wo_tiles = {}

            def load_wo(dc):
                wo_t = wop.tile([P, NQ, TB], bf16, tag="wo", bufs=3,
                                name=f"wo{dc}")
                nc.sync.dma_start(
                    wo_t[:], wo[:, :, dc * TB:(dc + 1) * TB]
                    .rearrange("h p d -> p h d"))
                wo_tiles[dc] = wo_t

            def ph3(tb, enc_hs, nxt, pp=None):
                copy_engs = [nc.scalar.copy, nc.scalar.copy]
                for dc in range(NDC):
                    if nxt is not None and dc == 4:
                        load_wq(2, nxt)
                    if nxt is not None and dc == 6:
                        load_wq(3, nxt)
                    if 2 <= dc + 2 < NDC:
                        load_wo(dc + 2)
                    wo_t = wo_tiles[dc]
                    for tci in range(TC):
                        if pp and dc == 0 and tci in pp:
                            # heads 0..5 pre-accumulated during attn(6)
                            yps = pp[tci]
                            h0 = NQ - 2
                        else:
                            yps = ps.tile([P, TB], f32, tag="big", bufs=6,
                                          name=f"yps{dc}_{tci}")
                            h0 = 0
                        for hh in range(h0, NQ):
                            nc.tensor.matmul(
                                yps[:],
                                enc_hs[hh][:, tci * P:(tci + 1) * P],
                                wo_t[:, hh, :],
                                start=hh == 0, stop=hh == NQ - 1)
                        ys = yp.tile([P, TB], bf16, tag="ys", bufs=3,
                                     name=f"ys{dc}_{tci}")
                        copy_engs[(dc * TC + tci) % 2](ys[:], yps[:])
                        nc.sync.dma_start(
                            y[tb * TB + tci * P:tb * TB + (tci + 1) * P,
                              dc * TB:(dc + 1) * TB], ys[:])

            # ---- startup DMA order: minimum needed for the first k/v
            # matmuls, then the rest ----
            x_tiles = {0: xp.tile([P, DT, TB], bf16, tag="x", name="x0")}
            x_cls0 = x_dma_closures(0, x_tiles[0], tables=False)
            nc.sync.dma_start(wk_sb[:, 0:2], wk_r[:, 0:2])
            nc.sync.dma_start(
                x_tiles[0][:, 0:2, :],
                xT[0:2 * P, 0:TB].rearrange("(g p) t -> p g t", p=P))
            nc.sync.dma_start(wk_sb[:, 2:4], wk_r[:, 2:4])
            nc.sync.dma_start(wv_sb[:, 0:4], wv_r[:, 0:4])
            nc.sync.dma_start(
                x_tiles[0][:, 2:4, :],
                xT[2 * P:4 * P, 0:TB].rearrange("(g p) t -> p g t", p=P))
            x_cls0[1]()
            for ch in range(1, 8):
                c4 = slice(ch * 4, (ch + 1) * 4)
                nc.sync.dma_start(wk_sb[:, c4], wk_r[:, c4])
                nc.sync.dma_start(wv_sb[:, c4], wv_r[:, c4])
                if ch + 1 < 8:
                    x_cls0[ch + 1]()
            nc.sync.dma_start(cos_sb[:, 0:TB], cos[:, 0:TB])
            nc.sync.dma_start(sin_sb[:, 0:TB], sin[:, 0:TB])
            nc.sync.dma_start(tri_sb[:], tri[:])
            load_wq(0, 0)
            load_wq(1, 0)

            # tb0 prologue: k/v projections. k streams per d-tile group as
            # x lands; v granules for chunks 0/2 interleave behind (their
            # d-tile ranges trail the x DMA), the rest follow dense.
            kc0, psk0 = kv_k_chunks(0)
            vcl, _ = kv_v_chunks(0)
            inter = [vcl[0], vcl[4], vcl[1], vcl[5], vcl[2], vcl[6], vcl[3]]
            kc0[0]()
            for g in range(1, 8):
                kc0[g]()
                inter[g - 1]()
            kv_finish_k(0, psk0)
            for b in vcl[7:]:
                b()

            kv_state = {}
            prim_state = {}

            def mk_rope(j, qts, tb):
                qts[j] = qtp.tile([P, TB], bf16, tag="qt", name=f"qt{j}")

                def tail():
                    rope(qts[j], psqs[j], tb)
                return tail

            for tb in range(NTB):
                enc_hs = [encp.tile([P, TB], bf16, tag="ench",
                                    name=f"ench{h}") for h in range(NQ)]
                qts = {}

                # heads 0 and 1: dense q-passes up front (depth-2 priming);
                # rope(0) after both passes, rope(1) deferred into attn(0)
                if tb == 0:
                    load_wq(2, tb)
                for ch in qchunks(0, tb):
                    ch()
                for ch in qchunks(1, tb):
                    ch()
                mk_rope(0, qts, tb)()
                rope1_fn = mk_rope(1, qts, tb)

                nxt = tb + 1 if tb + 1 < NTB else None
                fin = None
                pp = {}

                def pp_mm(tci, ha, hb, enc_hs=enc_hs, pp=pp):
                    def emit():
                        if tci not in pp:
                            pp[tci] = ps.tile([P, TB], f32, tag="big",
                                              bufs=6, name=f"ypsP{tci}")
                        for hh in range(ha, hb):
                            nc.tensor.matmul(
                                pp[tci][:],
                                enc_hs[hh][:, tci * P:(tci + 1) * P],
                                wo_tiles[0][:, hh, :],
                                start=hh == 0, stop=False)
                    return emit

                for h in range(NQ):
                    if h + 3 < NQ and (tb == 0 or h >= 1):
                        load_wq(h + 3, tb)
                    tail_fn = None
                    pe_fills = True
                    if h + 2 < NQ:
                        fills = qchunks(h + 2, tb)
                        tail_fn = mk_rope(h + 2, qts, tb)
                        if nxt is not None and h == 4:
                            # interleave next t-block's x prefetch
                            x_tiles[nxt] = xp.tile([P, DT, TB], bf16,
                                                   tag="x", name=f"x{nxt}")
                            xcl = x_dma_closures(nxt, x_tiles[nxt])
                            fills = [c for pair in zip(fills, xcl)
                                     for c in pair] + xcl[len(fills):]
                        elif nxt is not None and h == 5:
                            fills = fills + [
                                lambda tb=tb: load_wq(0, tb + 1),
                                lambda tb=tb: load_wq(1, tb + 1)]
                    elif h == NQ - 2:
                        # next t-block's k projections + first wo loads
                        fills = [lambda dc=dc: load_wo(dc) for dc in range(2)]
                        if nxt is not None:
                            kc, psk_n = kv_k_chunks(nxt)
                            kv_state["psk"] = psk_n
                            fills = kc + fills
                        else:
                            # last t-block: pre-accumulate heads 0..5 of
                            # ph3's first d-chunk to shorten the tail
                            fills += [pp_mm(0, 0, 3), pp_mm(1, 0, 3),
                                      pp_mm(0, 3, 6), pp_mm(1, 3, 6)]
                    else:
                        # last head: next t-block's v projections, k rope,
                        # v copies
                        fills = []
                        if nxt is not None:
                            vc, _ = kv_v_chunks(nxt)
                            fills = ([vc[0],
                                      lambda: kv_finish_k(nxt,
                                                          kv_state["psk"])] +
                                     vc[1:])
                        else:
                            pe_fills = False
                    fin = attn(h, tb, qts[h], enc_hs[h], fills, tail_fn,
                               pe_fills, head_fn=rope1_fn if h == 0 else None,
                               fin_prev=fin)
                fin()
                ph3(tb, enc_hs, nxt, pp)

    nc.compile()
    return nc


def _get_nc():
    if "nc" not in _STATE:
        _STATE["nc"] = _build_nc()
    return _STATE["nc"]


def _make_in_maps(x, positions, wq, wkv, wo):
    """Build the 8 per-core input dicts (host-side sharding + tables)."""
    B = x.shape[0]
    in_maps = []

    tables = []
    for b in range(B):
        pos = np.asarray(positions[b], np.float64)
        timescale = 10000.0 ** ((2.0 / H) * np.arange(H // 2))
        rad = pos[:, None] / timescale[None, :]          # [T, H/2]
        c64 = np.cos(rad).T                              # [H/2, T]
        s64 = np.sin(rad).T
        tables.append((
            np.ascontiguousarray(np.concatenate([c64, c64], 0)).astype(BF16),
            np.ascontiguousarray(np.concatenate([-s64, s64], 0)).astype(BF16),
        ))

    xTs = [np.ascontiguousarray(x[b].T).astype(BF16) for b in range(B)]

    M = np.zeros((P, P), np.float32)
    for h in range(H // 2):
        M[h, h + H // 2] = -1.0
        M[h + H // 2, h] = 1.0
    mrot = np.ascontiguousarray(M.T).astype(BF16)

    i = np.arange(P)[:, None]
    j = np.arange(P)[None, :]
    tri = np.ascontiguousarray((j >= i).astype(BF16))

    ones = np.ones((P, 1), np.float32)
    onesb = np.ones((P, 1), BF16)

    for c in range(8):
        b, hg = c // 4, c % 4
        qs = slice(NQ * hg, NQ * (hg + 1))
        ks = slice(NKV * hg, NKV * (hg + 1))
        cos_t, sin_t = tables[b]
        in_maps.append({
            "xT": xTs[b],
            "wq": np.ascontiguousarray(wq[qs].transpose(1, 0, 2)).astype(BF16),
            "wk": np.ascontiguousarray(wkv[0, ks].transpose(1, 0, 2)).astype(BF16),
            "wv": np.ascontiguousarray(wkv[1, ks].transpose(1, 0, 2)).astype(BF16),
            "wo": np.ascontiguousarray(wo[qs]).astype(BF16),
            "cos": cos_t,
            "sin": sin_t,
            "mrot": mrot,
            "tri": tri,
            "ones": ones,
            "onesb": onesb,
        })
    return in_maps


def run_cores(in_maps, trace=False, trace_cores=None):
    from concourse.bass_utils import run_bass_kernel_spmd
    nc = _get_nc()
    kw = {}
    if trace:
        kw = dict(trace=True,
                  trace_cores=trace_cores or list(range(8)))
    return run_bass_kernel_spmd(nc, in_maps, core_ids=list(range(8)), **kw)


def kernel(**inputs):
    x = np.asarray(inputs["x"], np.float32)
    positions = np.asarray(inputs["positions"])
    wq = np.asarray(inputs["wq"], np.float32)
    wkv = np.asarray(inputs["wkv"], np.float32)
    wo = np.asarray(inputs["wo"], np.float32)
    B = x.shape[0]
    assert x.shape == (2, T, D) and wq.shape == (32, D, H)

    in_maps = _make_in_maps(x, positions, wq, wkv, wo)
    res = run_cores(in_maps)
    y = np.zeros((B, T, D), np.float32)
    for c, r in enumerate(res.results):
        y[c // 4] += np.asarray(r["y"], np.float32)
    return y


if __name__ == "__main__":
    _build_nc()
    print("build OK")


# revision 67
# speedup vs baseline: 1.0034x; 1.0034x over previous
"""Trainium2 Bass kernel for GQA attention prefill (B=2, T=2048, D=4096, N=32, K=8, H=128).

Sharding: 8 cores = 2 (batch) x 4 (head-groups). Each core handles one batch
element, 8 q-heads and its 2 kv-heads, producing a partial output projection
(summed over its heads, bf16). Host sums the 4 partials per batch element.

Per-core pipeline, software-pipelined per head so PE never idles:
  passA(tb):  k,v projections from x_sb (x DMA'd once per t-block, kept
              resident in SBUF); rope(k) via a +-1 permutation matmul plus
              DVE elementwise with host cos/sin tables.
  per head h: q-projection matmuls for head h+1 are emitted interleaved with
              head h's attention s-block loop, so PE has dense independent
              work while ACT computes exp. Attention: logitsT [s128,t<=512]
              = kT-block @ qt (bf16, no transposes), exp on ACT, 0/1
              triangle mask multiply on DVE for diagonal tiles only, AV
              accumulates in PSUM. Softmax denominators: non-diagonal exp
              tiles accumulate on the Pool engine (off the critical path);
              one ones-vector matmul over that accumulator plus tiny per-tile
              ones-matmuls for the 4 diagonal blocks finish l on PE.
              During the last head, the next t-block's x / cos / sin DMAs
              are emitted so passA never waits on the queue.
  ph3(tb):    output projection from per-head enc tiles (no whole-tile dep),
              PSUM -> bf16 SBUF copies round-robined over DVE/Pool/ACT,
              DMA out per 512-wide d-chunk.
"""

import os
import sys

import numpy as np

for _p in ("/opt/trn_rl_repo", "/root/.axon_site/_ro/trn_rl_repo"):
    if _p not in sys.path and os.path.isdir(_p):
        sys.path.append(_p)

import ml_dtypes

BF16 = ml_dtypes.bfloat16

P = 128
T = 2048
D = 4096
H = 128
NQ = 8   # q heads per core
NKV = 2  # kv heads per core
TB = 512
NTB = T // TB        # 4
DT = D // P          # 32 d-tiles
NSB = T // P         # 16 s-blocks
TC = TB // P         # 4 t-chunks per t-block
NDC = D // TB        # 8 d-chunks for the output projection
SCALE = float(H) ** -0.5

_STATE = {}


def _build_nc():
    import concourse.mybir as mybir
    import concourse.tile as tile
    from concourse import bacc
    from concourse import bass_isa

    f32 = mybir.dt.float32
    f32r = mybir.dt.float32r
    bf16 = mybir.dt.bfloat16
    Alu = mybir.AluOpType
    Act = mybir.ActivationFunctionType

    nc = bacc.Bacc(None, target_bir_lowering=False, debug=False)

    xT = nc.dram_tensor("xT", [D, T], bf16, kind="ExternalInput")
    wq = nc.dram_tensor("wq", [D, NQ, H], bf16, kind="ExternalInput")
    wk = nc.dram_tensor("wk", [D, NKV, H], bf16, kind="ExternalInput")
    wv = nc.dram_tensor("wv", [D, NKV, H], bf16, kind="ExternalInput")
    wo = nc.dram_tensor("wo", [NQ, H, D], bf16, kind="ExternalInput")
    cos = nc.dram_tensor("cos", [P, T], bf16, kind="ExternalInput")
    sin = nc.dram_tensor("sin", [P, T], bf16, kind="ExternalInput")
    mrot = nc.dram_tensor("mrot", [P, P], bf16, kind="ExternalInput")
    tri = nc.dram_tensor("tri", [P, P], bf16, kind="ExternalInput")
    ones = nc.dram_tensor("ones", [P, 1], f32r, kind="ExternalInput")
    onesb = nc.dram_tensor("onesb", [P, 1], bf16, kind="ExternalInput")
    y = nc.dram_tensor("y", [T, D], bf16, kind="ExternalOutput")

    with tile.TileContext(nc) as tc:
        with (
            tc.tile_pool(name="const", bufs=1) as const,
            tc.tile_pool(name="xp", bufs=2) as xp,
            tc.tile_pool(name="wqp", bufs=2) as wqp,
            tc.tile_pool(name="qtp", bufs=3) as qtp,
            tc.tile_pool(name="rp", bufs=3) as rp,
            tc.tile_pool(name="ep", bufs=6) as ep,
            tc.tile_pool(name="eap", bufs=2) as eap,
            tc.tile_pool(name="encp", bufs=9) as encp,
            tc.tile_pool(name="lp", bufs=1) as lp,
            tc.tile_pool(name="wop", bufs=2) as wop,
            tc.tile_pool(name="yp", bufs=2) as yp,
            tc.tile_pool(name="ps", bufs=1, space="PSUM") as ps,
        ):
            wk_sb = const.tile([P, DT, NKV, H], bf16, tag="wk")
            wv_sb = const.tile([P, DT, NKV * H], bf16, tag="wv")
            wk_r = wk.rearrange("(dt p) h e -> p dt h e", p=P)
            wv_r = wv.rearrange("(dt p) h e -> p dt (h e)", p=P)
            tri_sb = const.tile([P, P], bf16, tag="tri")
            cos_sb = const.tile([P, T], bf16, tag="cos")
            sin_sb = const.tile([P, T], bf16, tag="sin")
            kT_all = const.tile([P, NKV, T], bf16, tag="kT")
            v_all = const.tile([P, NKV, NSB, H], bf16, tag="v")

            def x_dma_closures(tb, x_sb, tables=True):
                """8 closures, each DMA-ing a 4-d-tile chunk of x for tb
                (plus this t-block's cos/sin on the first chunk)."""
                tsl = slice(tb * TB, (tb + 1) * TB)

                def mk(c8):
                    def emit():
                        nc.sync.dma_start(
                            x_sb[:, c8 * 4:(c8 + 1) * 4, :],
                            xT[c8 * 4 * P:(c8 + 1) * 4 * P, tsl]
                            .rearrange("(g p) t -> p g t", p=P))
                        if c8 == 0 and tables:
                            nc.sync.dma_start(cos_sb[:, tsl], cos[:, tsl])
                            nc.sync.dma_start(sin_sb[:, tsl], sin[:, tsl])
                    return emit

                return [mk(c8) for c8 in range(8)]

            def rope(dst, src_ps, tb):
                """dst[:] = rope(src_ps) for one head's [H, TB] block.
                Half-swap via SBUF->SBUF DMA partition reorder (no PE);
                the sign of the swapped half is folded into the sin table
                ([-sin; +sin])."""
                cs = cos_sb[:, tb * TB:(tb + 1) * TB]
                sn = sin_sb[:, tb * TB:(tb + 1) * TB]
                raw = rp.tile([P, TB], bf16, tag="raw", name="raw")
                nc.scalar.copy(raw[:], src_ps[:])
                shuf = rp.tile([P, TB], bf16, tag="shuf", name="shuf")
                nc.sync.dma_start(shuf[0:P // 2, :], raw[P // 2:P, :])
                nc.sync.dma_start(shuf[P // 2:P, :], raw[0:P // 2, :])
                tmp = rp.tile([P, TB], bf16, tag="tmp", name="tmp")
                nc.vector.tensor_tensor(dst, raw[:], cs, Alu.mult)
                nc.vector.tensor_tensor(tmp[:], shuf[:], sn, Alu.mult)
                nc.vector.tensor_tensor(dst, dst, tmp[:], Alu.add)

            def kv_k_chunks(tbx):
                """k-projection matmul chunk closures (+ psk tiles) for tbx."""
                x_sb = x_tiles[tbx]
                psk = [ps.tile([P, TB], f32, tag="big", bufs=6,
                               name=f"psk{_k}") for _k in range(NKV)]

                def mk(d0, d1):
                    def emit():
                        for dt in range(d0, d1):
                            for kk in range(NKV):
                                nc.tensor.matmul(
                                    psk[kk][:], wk_sb[:, dt, kk, :],
                                    x_sb[:, dt, :],
                                    start=dt == 0, stop=dt == DT - 1)
                    return emit

                return [mk(d, d + 4) for d in range(0, DT, 4)], psk

            def kv_v_chunks(tbx):
                """v-projection closures for tbx. Each of the 4 t-chunks gets
                its OWN PSUM tile (one accumulation group per bank — two
                groups in one bank is illegal), sequenced so a chunk is
                copied out to v_all before its bank slot is reused."""
                x_sb = x_tiles[tbx]
                state = {}

                def mk_mm(c, d0, d1):
                    def emit():
                        if d0 == 0:
                            state[c] = ps.tile([P, NKV * H], f32, tag="big",
                                               bufs=6, name=f"psv{c}")
                        for dt in range(d0, d1):
                            nc.tensor.matmul(
                                state[c][:],
                                x_sb[:, dt, c * P:(c + 1) * P],
                                wv_sb[:, dt, :],
                                start=dt == 0, stop=dt == DT - 1)
                    return emit

                def mk_copy(c):
                    def emit():
                        nc.scalar.copy(
                            v_all[:, :, tbx * TC + c, :],
                            state[c][:].rearrange("p (h e) -> p h e", h=NKV))
                    return emit

                chunks = []
                for c in (0, 2):
                    chunks += [mk_mm(c, d, d + 8) for d in range(0, DT, 8)]
                chunks.append(mk_copy(0))
                chunks += [mk_mm(1, d, d + 8) for d in range(0, DT, 8)]
                chunks.append(mk_copy(2))
                chunks += [mk_mm(3, d, d + 8) for d in range(0, DT, 8)]
                chunks.append(mk_copy(1))
                chunks.append(mk_copy(3))
                return chunks, None

            def kv_finish_k(tbx, psk):
                tsl = slice(tbx * TB, (tbx + 1) * TB)
                for kk in range(NKV):
                    rope(kT_all[:, kk, tsl], psk[kk], tbx)

            wqs = {}
            psqs = {}

            def load_wq(h, tb):
                wq_t = wqp.tile([P, DT, H], bf16, tag="wq", name=f"wq{h}")
                for c in range(4):
                    nc.sync.dma_start(
                        wq_t[:, c * 8:(c + 1) * 8, :],
                        wq[c * 8 * P:(c + 1) * 8 * P, h, :]
                        .rearrange("(dt p) e -> p dt e", p=P))
                wqs[h] = wq_t

            def qchunks(h, tb):
                """Closures each emitting a few of head h's 32 accumulating
                q-projection matmuls (wq must already be loading)."""
                x_sb = x_tiles[tb]
                wq_t = wqs[h]
                psq = ps.tile([P, TB], f32, tag="big", bufs=6, name=f"psq{h}")
                psqs[h] = psq

                def mk(d0, d1):
                    def emit():
                        for dt in range(d0, d1):
                            nc.tensor.matmul(psq[:], wq_t[:, dt, :],
                                             x_sb[:, dt, :],
                                             start=dt == 0, stop=dt == DT - 1)
                    return emit

                return [mk(d, min(d + 4, DT)) for d in range(0, DT, 4)]

            def attn(h, tb, qt, enc_h, fills, tail_fn, pe_fills=True,
                     head_fn=None, fin_prev=None):
                """Attention for head h. `fills` are closures interleaved into
                the s-block loop (a later head's q matmuls, or DMA prefetch);
                `tail_fn` (that head's rope) is emitted once fills are done."""
                nsb = TC * (tb + 1)
                ndg = nsb - TC  # non-diagonal s-block count
                kk = h // 4
                enc_ps = ps.tile([P, TB], f32, tag="acc", bufs=2,
                                 name=f"encps{h}")
                exacc = eap.tile([P, TB], bf16, tag="eacc", name=f"eacc{h}")
                look = 5
                ex = [None] * nsb
                tail_state = {"done": tail_fn is None}

                def pre(sb):
                    r = sb - ndg
                    off = P * r if r >= 0 else 0
                    csl = slice(off, TB)
                    lg = ps.tile([P, TB], f32, tag="big", bufs=6,
                                 name=f"lg{sb}")
                    nc.tensor.matmul(lg[:, csl],
                                     kT_all[:, kk, sb * P:(sb + 1) * P],
                                     qt[:, csl])
                    ex_t = ep.tile([P, TB], bf16, tag="ex", name=f"ex{sb}")
                    nc.scalar.activation(ex_t[:, csl], lg[:, csl], Act.Exp,
                                         scale=SCALE)
                    if r >= 0:
                        nc.vector.tensor_tensor(ex_t[:, off:off + P],
                                                ex_t[:, off:off + P],
                                                tri_sb[:], Alu.mult)
                    ex[sb] = (ex_t, csl)

                def post(sb):
                    ex_t, csl = ex[sb]
                    nc.tensor.matmul(enc_ps[:, csl], v_all[:, kk, sb, :],
                                     ex_t[:, csl],
                                     start=sb == 0, stop=sb == nsb - 1)
                    # exp-tile accumulation for the softmax denominator (DVE)
                    if sb == 0:
                        nc.vector.tensor_copy(exacc[:], ex_t[:])
                    else:
                        nc.vector.tensor_tensor(exacc[:, csl], exacc[:, csl],
                                                ex_t[:, csl], Alu.add)

                ci = 0
                while ci < min(2, len(fills)):
                    fills[ci]()
                    ci += 1
                if head_fn is not None:
                    head_fn()
                for sb in range(min(look, nsb)):
                    pre(sb)
                if fin_prev is not None:
                    fin_prev()
                for sb in range(nsb):
                    want = min(len(fills),
                               max((sb + 3) * len(fills) // nsb, 3))
                    while ci < want:
                        fills[ci]()
                        ci += 1
                    if ci == len(fills) and not tail_state["done"]:
                        tail_fn()
                        tail_state["done"] = True
                    post(sb)
                    if sb + look < nsb:
                        pre(sb + look)
                while ci < len(fills):
                    fills[ci]()
                    ci += 1
                if not tail_state["done"]:
                    tail_fn()

                def finalize():
                    lsum = lp.tile([P, TB], f32, tag="lsum", name="lsum")
                    nc.gpsimd.partition_all_reduce(lsum[:], exacc[:], P,
                                                   bass_isa.ReduceOp.add)
                    rinv = lp.tile([P, TB], f32, tag="rinv", name="rinv")
                    nc.vector.reciprocal(rinv[:], lsum[:])
                    nc.vector.tensor_tensor(enc_h[:], enc_ps[:], rinv[:],
                                            Alu.mult)
                return finalize

            wo_tiles = {}

            def load_wo(dc):
                wo_t = wop.tile([P, NQ, TB], bf16, tag="wo", bufs=3,
                                name=f"wo{dc}")
                nc.sync.dma_start(
                    wo_t[:], wo[:, :, dc * TB:(dc + 1) * TB]
                    .rearrange("h p d -> p h d"))
                wo_tiles[dc] = wo_t

            def ph3(tb, enc_hs, nxt, pp=None):
                copy_engs = [nc.scalar.copy, nc.scalar.copy]
                for dc in range(NDC):
                    if nxt is not None and dc == 4:
                        load_wq(2, nxt)
                    if nxt is not None and dc == 6:
                        load_wq(3, nxt)
                    if 2 <= dc + 2 < NDC:
                        load_wo(dc + 2)
                    wo_t = wo_tiles[dc]
                    for tci in range(TC):
                        if pp and dc == 0 and tci in pp:
                            # heads 0..5 pre-accumulated during attn(6)
                            yps = pp[tci]
                            h0 = NQ - 2
                        else:
                            yps = ps.tile([P, TB], f32, tag="big", bufs=6,
                                          name=f"yps{dc}_{tci}")
                            h0 = 0
                        for hh in range(h0, NQ):
                            nc.tensor.matmul(
                                yps[:],
                                enc_hs[hh][:, tci * P:(tci + 1) * P],
                                wo_t[:, hh, :],
                                start=hh == 0, stop=hh == NQ - 1)
                        ys = yp.tile([P, TB], bf16, tag="ys", bufs=3,
                                     name=f"ys{dc}_{tci}")
                        copy_engs[(dc * TC + tci) % 2](ys[:], yps[:])
                        nc.sync.dma_start(
                            y[tb * TB + tci * P:tb * TB + (tci + 1) * P,
                              dc * TB:(dc + 1) * TB], ys[:])

            # ---- startup DMA order: minimum needed for the first k/v
            # matmuls, then the rest ----
            x_tiles = {0: xp.tile([P, DT, TB], bf16, tag="x", name="x0")}
            x_cls0 = x_dma_closures(0, x_tiles[0], tables=False)
            nc.sync.dma_start(wk_sb[:, 0:2], wk_r[:, 0:2])
            nc.sync.dma_start(
                x_tiles[0][:, 0:2, :],
                xT[0:2 * P, 0:TB].rearrange("(g p) t -> p g t", p=P))
            nc.sync.dma_start(wk_sb[:, 2:4], wk_r[:, 2:4])
            nc.sync.dma_start(wv_sb[:, 0:4], wv_r[:, 0:4])
            nc.sync.dma_start(
                x_tiles[0][:, 2:4, :],
                xT[2 * P:4 * P, 0:TB].rearrange("(g p) t -> p g t", p=P))
            x_cls0[1]()
            for ch in range(1, 8):
                c4 = slice(ch * 4, (ch + 1) * 4)
                nc.sync.dma_start(wk_sb[:, c4], wk_r[:, c4])
                nc.sync.dma_start(wv_sb[:, c4], wv_r[:, c4])
                if ch + 1 < 8:
                    x_cls0[ch + 1]()
            nc.sync.dma_start(cos_sb[:, 0:TB], cos[:, 0:TB])
            nc.sync.dma_start(sin_sb[:, 0:TB], sin[:, 0:TB])
            nc.sync.dma_start(tri_sb[:], tri[:])
            load_wq(0, 0)
            load_wq(1, 0)

            # tb0 prologue: k/v projections. k streams per d-tile group as
            # x lands; v granules for chunks 0/2 interleave behind (their
            # d-tile ranges trail the x DMA), the rest follow dense.
            kc0, psk0 = kv_k_chunks(0)
            vcl, _ = kv_v_chunks(0)
            inter = [vcl[0], vcl[4], vcl[1], vcl[5], vcl[2], vcl[6], vcl[3]]
            kc0[0]()
            for g in range(1, 8):
                kc0[g]()
                inter[g - 1]()
            kv_finish_k(0, psk0)
            for b in vcl[7:]:
                b()

            kv_state = {}
            prim_state = {}

            def mk_rope(j, qts, tb):
                qts[j] = qtp.tile([P, TB], bf16, tag="qt", name=f"qt{j}")

                def tail():
                    rope(qts[j], psqs[j], tb)
                return tail

            for tb in range(NTB):
                enc_hs = [encp.tile([P, TB], bf16, tag="ench",
                                    name=f"ench{h}") for h in range(NQ)]
                qts = {}

                # heads 0 and 1: dense q-passes up front (depth-2 priming);
                # rope(0) after both passes, rope(1) deferred into attn(0)
                if tb == 0:
                    load_wq(2, tb)
                for ch in qchunks(0, tb):
                    ch()
                for ch in qchunks(1, tb):
                    ch()
                mk_rope(0, qts, tb)()
                rope1_fn = mk_rope(1, qts, tb)

                nxt = tb + 1 if tb + 1 < NTB else None
                fin = None
                pp = {}

                def pp_mm(tci, ha, hb, enc_hs=enc_hs, pp=pp):
                    def emit():
                        if tci not in pp:
                            pp[tci] = ps.tile([P, TB], f32, tag="big",
                                              bufs=6, name=f"ypsP{tci}")
                        for hh in range(ha, hb):
                            nc.tensor.matmul(
                                pp[tci][:],
                                enc_hs[hh][:, tci * P:(tci + 1) * P],
                                wo_tiles[0][:, hh, :],
                                start=hh == 0, stop=False)
                    return emit

                for h in range(NQ):
                    if h + 3 < NQ and (tb == 0 or h >= 1):
                        load_wq(h + 3, tb)
                    tail_fn = None
                    pe_fills = True
                    if h + 2 < NQ:
                        fills = qchunks(h + 2, tb)
                        tail_fn = mk_rope(h + 2, qts, tb)
                        if nxt is not None and h == 4:
                            # interleave next t-block's x prefetch
                            x_tiles[nxt] = xp.tile([P, DT, TB], bf16,
                                                   tag="x", name=f"x{nxt}")
                            xcl = x_dma_closures(nxt, x_tiles[nxt])
                            fills = [c for pair in zip(fills, xcl)
                                     for c in pair] + xcl[len(fills):]
                        elif nxt is not None and h == 5:
                            fills = fills + [
                                lambda tb=tb: load_wq(0, tb + 1),
                                lambda tb=tb: load_wq(1, tb + 1)]
                    elif h == NQ - 2:
                        # next t-block's k projections + first wo loads
                        fills = [lambda dc=dc: load_wo(dc) for dc in range(2)]
                        if nxt is not None:
                            kc, psk_n = kv_k_chunks(nxt)
                            kv_state["psk"] = psk_n
                            fills = kc + fills
                        else:
                            # last t-block: pre-accumulate heads 0..5 of
                            # ph3's first d-chunk to shorten the tail
                            fills += [pp_mm(0, 0, 3), pp_mm(1, 0, 3),
                                      pp_mm(0, 3, 6), pp_mm(1, 3, 6)]
                    else:
                        # last head: next t-block's v projections, k rope,
                        # v copies
                        fills = []
                        if nxt is not None:
                            vc, _ = kv_v_chunks(nxt)
                            fills = ([vc[0],
                                      lambda: kv_finish_k(nxt,
                                                          kv_state["psk"])] +
                                     vc[1:])
                        else:
                            pe_fills = False
                    fin = attn(h, tb, qts[h], enc_hs[h], fills, tail_fn,
                               pe_fills, head_fn=rope1_fn if h == 0 else None,
                               fin_prev=fin)
                fin()
                ph3(tb, enc_hs, nxt, pp)

    nc.compile()
    return nc


def _get_nc():
    if "nc" not in _STATE:
        _STATE["nc"] = _build_nc()
    return _STATE["nc"]


def _make_in_maps(x, positions, wq, wkv, wo):
    """Build the 8 per-core input dicts (host-side sharding + tables)."""
    B = x.shape[0]
    in_maps = []

    tables = []
    for b in range(B):
        pos = np.asarray(positions[b], np.float64)
        timescale = 10000.0 ** ((2.0 / H) * np.arange(H // 2))
        rad = pos[:, None] / timescale[None, :]          # [T, H/2]
        c64 = np.cos(rad).T                              # [H/2, T]
        s64 = np.sin(rad).T
        tables.append((
            np.ascontiguousarray(np.concatenate([c64, c64], 0)).astype(BF16),
            np.ascontiguousarray(np.concatenate([-s64, s64], 0)).astype(BF16),
        ))

    xTs = [np.ascontiguousarray(x[b].T).astype(BF16) for b in range(B)]

    M = np.zeros((P, P), np.float32)
    for h in range(H // 2):
        M[h, h + H // 2] = -1.0
        M[h + H // 2, h] = 1.0
    mrot = np.ascontiguousarray(M.T).astype(BF16)

    i = np.arange(P)[:, None]
    j = np.arange(P)[None, :]
    tri = np.ascontiguousarray((j >= i).astype(BF16))

    ones = np.ones((P, 1), np.float32)
    onesb = np.ones((P, 1), BF16)

    for c in range(8):
        b, hg = c // 4, c % 4
        qs = slice(NQ * hg, NQ * (hg + 1))
        ks = slice(NKV * hg, NKV * (hg + 1))
        cos_t, sin_t = tables[b]
        in_maps.append({
            "xT": xTs[b],
            "wq": np.ascontiguousarray(wq[qs].transpose(1, 0, 2)).astype(BF16),
            "wk": np.ascontiguousarray(wkv[0, ks].transpose(1, 0, 2)).astype(BF16),
            "wv": np.ascontiguousarray(wkv[1, ks].transpose(1, 0, 2)).astype(BF16),
            "wo": np.ascontiguousarray(wo[qs]).astype(BF16),
            "cos": cos_t,
            "sin": sin_t,
            "mrot": mrot,
            "tri": tri,
            "ones": ones,
            "onesb": onesb,
        })
    return in_maps


def run_cores(in_maps, trace=False, trace_cores=None):
    from concourse.bass_utils import run_bass_kernel_spmd
    nc = _get_nc()
    kw = {}
    if trace:
        kw = dict(trace=True,
                  trace_cores=trace_cores or list(range(8)))
    return run_bass_kernel_spmd(nc, in_maps, core_ids=list(range(8)), **kw)


def kernel(**inputs):
    x = np.asarray(inputs["x"], np.float32)
    positions = np.asarray(inputs["positions"])
    wq = np.asarray(inputs["wq"], np.float32)
    wkv = np.asarray(inputs["wkv"], np.float32)
    wo = np.asarray(inputs["wo"], np.float32)
    B = x.shape[0]
    assert x.shape == (2, T, D) and wq.shape == (32, D, H)

    in_maps = _make_in_maps(x, positions, wq, wkv, wo)
    res = run_cores(in_maps)
    y = np.zeros((B, T, D), np.float32)
    for c, r in enumerate(res.results):
        y[c // 4] += np.asarray(r["y"], np.float32)
    return y


if __name__ == "__main__":
    _build_nc()
    print("build OK")
